# revision 20
# baseline (speedup 1.0000x reference)
"""Trainium2 Bass kernel for nn_Attention_46420006535531 (v2).

Gated multi-head attention with additive attention bias:
    q = x@Wq, (k, v) = split(x@Wkv), heads=8, dim_head=64
    attn = softmax(q*k^T*scale + bias); out = attn@v
    out = (out * sigmoid(x@Wg + bg)) @ Wo + bo

Sharding: 8 cores; core c handles batch b=c//2 and the 4 heads
4*(c%2)..4*(c%2)+3.  Each core computes a partial y (its heads' slice
of Wo rows); the host sums the two partials per batch and adds bo.

Layout notes (all on-core data transposed, fp16 pipeline):
 - S^T[j,i] per head in [128,512] tiles; two tiles share one
   [128,1024] PSUM buffer so ACT exps 1024 elements per instruction.
 - bias enters as exp(bias)^T fp16 (host-prepped); attention weights
   are exp(S)*exp(bias) via fp16 DVE/Pool muls.
 - AV runs as TWO parallel PSUM accumulation chains (even/odd j) so
   same-chain accumulates are 2 PE instructions apart - the PSUM
   read-modify-write turnaround (~500ns on HW) stays hidden.  The
   chains are summed into SBUF fp16 by the tail, which also yields the
   softmax denominator (ones column rides row 64 of the v tiles).
 - reciprocal is broadcast to 64 partitions by a [1,64]x[1,512] matmul
   of 0.5-constants; the 0.5 folds the tanh gate identity
   sigmoid(z) = 0.5 + 0.5*tanh(z/2), so gates use Tanh and stay in the
   same ACT table as Exp - no per-iteration ACT table reloads.
 - projections are emission-interleaved into the first attention steps
   to keep ACT busy across the iteration boundary; gates/out-proj pack
   two heads along 128 partitions (full PE rows).
 - y partials leave as fp16; all DMA goes through sync/HWDGE (Pool
   cannot touch PSUM and Pool-issued DMA costs Pool engine time).
"""
import sys
import numpy as np

for _p in ("/opt/trn_rl_repo",):
    if _p not in sys.path:
        sys.path.insert(0, _p)

import concourse.bass as bass
import concourse.bacc as bacc
import concourse.tile as tile
from concourse import mybir
from concourse.bass_utils import run_bass_kernel_spmd

B, N, DIM = 4, 1024, 256
HEADS, DIM_HEAD, INNER = 8, 64, 512
HPC = 4                      # heads per core
NCORES = 8
SCALE = DIM_HEAD ** -0.5     # folded into Wq on the host

F32 = mybir.dt.float32
FP16 = mybir.dt.float16
AF = mybir.ActivationFunctionType
ALU = mybir.AluOpType

NB = N // 512                # 2 i-blocks of 512
NJP = N // 128               # 8 j partition tiles
KK = DIM // 128              # 2 k-tiles for the projections


def _build_program(reps=1, loop_iters=0, static_bias=False):
    nc = bacc.Bacc(None, target_bir_lowering=False)

    # ---- DRAM I/O (per core) ----
    xt_d = nc.dram_tensor("xt", [128, KK, N], FP16, kind="ExternalInput")
    bias_d = nc.dram_tensor("bias_t", [HPC, NB, 128, NJP * 512], FP16,
                            kind="ExternalInput")
    wq_d = nc.dram_tensor("wq", [128, KK, 256], FP16, kind="ExternalInput")
    wk_d = nc.dram_tensor("wk", [128, KK, 256], FP16, kind="ExternalInput")
    wv_d = nc.dram_tensor("wv", [128, KK, 256], FP16, kind="ExternalInput")
    wg_d = nc.dram_tensor("wg", [128, KK, 256], FP16, kind="ExternalInput")
    bgh_d = nc.dram_tensor("bgh", [64, HPC], F32, kind="ExternalInput")
    wo_d = nc.dram_tensor("wo", [2, 128, 256], FP16, kind="ExternalInput")
    hv_d = nc.dram_tensor("halves64", [1, 64], FP16, kind="ExternalInput")
    id_d = nc.dram_tensor("ident", [128, 128], FP16, kind="ExternalInput")
    y_d = nc.dram_tensor("y", [N, 256], FP16, kind="ExternalOutput")

    with tile.TileContext(nc) as tc:
        import contextlib
        with contextlib.ExitStack() as ctx:
            const = ctx.enter_context(tc.tile_pool(name="const", bufs=1))
            acts = ctx.enter_context(tc.tile_pool(name="acts", bufs=2))
            biasp = ctx.enter_context(tc.tile_pool(name="biasp", bufs=6))
            pexp = ctx.enter_context(tc.tile_pool(name="pexp", bufs=6))
            pmul = ctx.enter_context(tc.tile_pool(name="pmul", bufs=6))
            small = ctx.enter_context(tc.tile_pool(name="small", bufs=4))
            ps_big = ctx.enter_context(tc.tile_pool(name="ps_big", bufs=2, space="PSUM"))
            ps_o = ctx.enter_context(tc.tile_pool(name="ps_o", bufs=3, space="PSUM"))
            ps_m = ctx.enter_context(tc.tile_pool(name="ps_m", bufs=1, space="PSUM"))

            # ---- constants / weights into SBUF (once) ----
            halves = const.tile([1, 64], FP16, tag="halves64")
            nc.sync.dma_start(out=halves[:], in_=hv_d[:])
            bgh_sb = const.tile([64, HPC], F32, tag="bgh")
            nc.sync.dma_start(out=bgh_sb[:], in_=bgh_d[:])
            ident = const.tile([128, 128], FP16, tag="ident")
            nc.sync.dma_start(out=ident[:], in_=id_d[:])
            wq_sb = const.tile([128, KK, 256], FP16, tag="wq")
            nc.sync.dma_start(out=wq_sb[:], in_=wq_d[:])
            wk_sb = const.tile([128, KK, 256], FP16, tag="wk")
            nc.sync.dma_start(out=wk_sb[:], in_=wk_d[:])
            wv_sb = const.tile([128, KK, 256], FP16, tag="wv")
            nc.sync.dma_start(out=wv_sb[:], in_=wv_d[:])
            wg_sb = const.tile([128, KK, 256], FP16, tag="wg")
            nc.sync.dma_start(out=wg_sb[:], in_=wg_d[:])
            wo_sb = []
            for p in range(2):
                t = const.tile([128, 256], FP16, tag=f"wo{p}")
                nc.sync.dma_start(out=t[:], in_=wo_d[p])
                wo_sb.append(t)
            xt_sb = const.tile([128, KK, N], FP16, tag="xt")
            nc.sync.dma_start(out=xt_sb[:], in_=xt_d[:])

            static_bt = None
            if static_bias:
                static_bt = []
                for st in range(NB * HPC):
                    ib, h = st // HPC, st % HPC
                    sbt = const.tile([128, NJP, 512], FP16, tag=f"sbt{st}")
                    nc.sync.dma_start(
                        out=sbt[:],
                        in_=bias_d[h, ib].rearrange("p (j n) -> p j n", j=NJP))
                    static_bt.append(sbt)

            env = dict(locals())
            lp = nc.allow_low_precision(reason="fp16 attention pipeline")
            lp.__enter__()

            if loop_iters:
                with tc.For_i(0, loop_iters, 1):
                    _emit_body(nc, tc, env)
            else:
                for _rep in range(reps):
                    _emit_body(nc, tc, env)

            lp.__exit__(None, None, None)

    nc.compile()
    return nc


def _emit_body(nc, tc, env):
    acts = env["acts"]; biasp = env["biasp"]
    pexp = env["pexp"]; pmul = env["pmul"]; small = env["small"]
    ps_big = env["ps_big"]; ps_o = env["ps_o"]; ps_m = env["ps_m"]
    halves = env["halves"]; bgh_sb = env["bgh_sb"]; ident = env["ident"]
    wq_sb = env["wq_sb"]; wk_sb = env["wk_sb"]; wv_sb = env["wv_sb"]
    wg_sb = env["wg_sb"]; wo_sb = env["wo_sb"]; xt_sb = env["xt_sb"]
    bias_d = env["bias_d"]; y_d = env["y_d"]

    # Per-j-pair bias strategy: jp0 adds raw bias into the S PSUM tile via
    # identity matmuls on PE (no elementwise op); jp1's exp(bias) mul goes
    # to Pool (its slow mul is consumed last via SLOT_JJ); jp2 muls on DVE;
    # jp3 alternates PE/DVE by head parity to balance those two engines.
    def jp_class(jp, h):
        if jp == 0:
            return "pe"
        if jp == 1:
            return "pool"
        if jp == 3 and h % 2 == 0:
            return "pe"
        return "dve"
    SLOT_JJ = (0, 1, 4, 5, 6, 7, 2, 3)

    # ---- projection helpers (emission interleaved into the steps) ----
    qT, kT = [], []
    for p in range(2):
        qt = acts.tile([128, N], FP16, tag=f"qT{p}")
        kt = acts.tile([128, N], FP16, tag=f"kT{p}")
        qT.append(qt)
        kT.append(kt)

    th4 = [[None] * NB for _ in range(HPC)]

    def emit_psg(p, ib, part):
        """part 0: matmuls; part 1: two per-head tanh instrs (base 0)."""
        if part == 0:
            psg = ps_m.tile([128, 512], F32, tag="misc")
            for kk in range(KK):
                nc.tensor.matmul(
                    psg[:], lhsT=wg_sb[:, kk, 128 * p:128 * p + 128],
                    rhs=xt_sb[:, kk, 512 * ib:512 * ib + 512],
                    start=(kk == 0), stop=(kk == KK - 1))
            emit_psg.ps[(p, ib)] = psg
        else:
            psg = emit_psg.ps[(p, ib)]
            for half in range(2):
                h = 2 * p + half
                gt = acts.tile([64, 512], FP16, tag=f"th{h}{ib}")
                nc.scalar.activation(
                    gt[:], psg[64 * half:64 * half + 64, :], AF.Tanh,
                    bias=bgh_sb[:, h:h + 1], scale=0.5)
                th4[h][ib] = gt
    emit_psg.ps = {}

    # qk pair-1 emission pieces: 8 matmuls into 4 ps_m tiles + 4 copies
    def qk1_mm(idx):
        # idx 0..7: (q/k, ib, kk) = (idx//4, (idx//2)%2, idx%2)
        qk, ib, kk = idx // 4, (idx // 2) % 2, idx % 2
        w = wq_sb if qk == 0 else wk_sb
        if kk == 0:
            t = ps_m.tile([128, 512], F32, tag="misc")
            qk1_mm.ps[(qk, ib)] = t
        nc.tensor.matmul(
            qk1_mm.ps[(qk, ib)][:], lhsT=w[:, kk, 128:256],
            rhs=xt_sb[:, kk, 512 * ib:512 * ib + 512],
            start=(kk == 0), stop=(kk == KK - 1))
        if kk == KK - 1:
            dst = qT[1] if qk == 0 else kT[1]
            nc.vector.tensor_copy(dst[:, 512 * ib:512 * ib + 512],
                                  qk1_mm.ps[(qk, ib)][:])
    qk1_mm.ps = {}

    # ---- preamble: v projections + qk pair-0, interleaved ----
    vaug = []
    for jp in range(NJP):
        vt = acts.tile([128, HPC, 65], FP16, tag=f"vaug{jp}")
        vaug.append(vt)
        nc.gpsimd.memset(vt[:, :, 64], 1.0)
    psv = [None] * NJP
    psqk0 = [None] * NB
    for jph in range(4):            # pairs of jp
        for jp in (2 * jph, 2 * jph + 1):
            if jp % 2 == 0:
                t = ps_m.tile([128, 256], F32, tag="misc")
            else:
                t = ps_o.tile([128, 256], F32, tag="po")
            psv[jp] = t
            for kk in range(KK):
                nc.tensor.matmul(
                    t[:], lhsT=xt_sb[:, kk, 128 * jp:128 * jp + 128],
                    rhs=wv_sb[:, kk, :],
                    start=(kk == 0), stop=(kk == KK - 1))
        # two qk-pair0 matmuls per jp-pair: jph indexes (q/k, ib)
        qk, ib = jph // 2, jph % 2
        if qk == 0 and ib == 0:
            for b2 in range(NB):
                t = ps_big.tile([128, 1024], F32, tag="big")
                psqk0[b2] = t
        for kk in range(KK):
            w = wq_sb if qk == 0 else wk_sb
            nc.tensor.matmul(
                psqk0[ib][:, 512 * qk:512 * qk + 512],
                lhsT=w[:, kk, 0:128],
                rhs=xt_sb[:, kk, 512 * ib:512 * ib + 512],
                start=(kk == 0), stop=(kk == KK - 1))
        for jp in (2 * jph, 2 * jph + 1):
            nc.vector.tensor_copy(
                vaug[jp][:, :, 0:64],
                psv[jp][:].rearrange("p (h d) -> p h d", h=HPC))
        if jph == 1:                # q done for both ib
            for b2 in range(NB):
                nc.vector.tensor_copy(qT[0][:, 512 * b2:512 * b2 + 512],
                                      psqk0[b2][:, 0:512])
        if jph == 3:                # k done for both ib
            for b2 in range(NB):
                nc.vector.tensor_copy(kT[0][:, 512 * b2:512 * b2 + 512],
                                      psqk0[b2][:, 512:1024])

    # interleave schedule: step -> slot -> list of emission thunks
    inter = {0: {j: [lambda j=j: qk1_mm(j)] for j in range(NJP)},
             1: {0: [lambda: emit_psg(0, 0, 0)],
                 1: [lambda: emit_psg(0, 0, 1)],
                 2: [lambda: emit_psg(1, 0, 0)],
                 3: [lambda: emit_psg(1, 0, 1)]},
             2: {0: [lambda: emit_psg(0, 1, 0)],
                 1: [lambda: emit_psg(0, 1, 1)]},
             3: {0: [lambda: emit_psg(1, 1, 0)],
                 1: [lambda: emit_psg(1, 1, 1)]}}

    # ---- attention steps ----
    # Steps (ib, h); step s's qk/exp/mul interleave 1:1 with step s-1's
    # AV chain.
    steps = [(ib, h) for ib in range(NB) for h in range(HPC)]
    og_by_ib = [[None] * HPC for _ in range(NB)]
    og_tiles = {}
    prev = None
    mul_ctr = [0]

    def emit_tail(st):
        ib, h = st["ib"], st["h"]
        poA, poB = st["poA"], st["poB"]
        pair, half = h // 2, h % 2
        poBc = small.tile([65, 512], FP16, tag="poBc")
        nc.vector.tensor_copy(poBc[:], poB[:])
        poS = small.tile([65, 512], FP16, tag="poS")
        nc.vector.tensor_add(poS[:], poA[:], poBc[:])
        r = small.tile([1, 512], FP16, tag="recip")
        nc.vector.reciprocal(r[:], poS[64:65, :])
        pR = ps_m.tile([64, 512], F32, tag="misc")
        nc.tensor.matmul(pR[:], lhsT=halves[:], rhs=r[:],
                         start=True, stop=True)
        t1 = small.tile([64, 512], FP16, tag="t1")
        nc.vector.scalar_tensor_tensor(
            t1[:], th4[h][ib][:], 1.0,
            poS[0:64, :], ALU.add, ALU.mult)
        key = (ib, pair)
        if key not in og_tiles:
            og = acts.tile([128, 512], FP16, tag=f"og{ib}{pair}")
            og_tiles[key] = og
        og = og_tiles[key]
        nc.vector.tensor_mul(og[64 * half:64 * half + 64, :], t1[:], pR[:])
        og_by_ib[ib][h] = og
        if h == HPC - 1:
            # out-proj: psy tiles from ps_big (free between exp batches);
            # RMW partner (pr=1) spaced 2+ from its pr=0 matmul.
            for icg in range(2):          # groups of 2 ic
                psys = []
                for ic in (2 * icg, 2 * icg + 1):
                    psy = ps_big.tile([128, 256], F32, tag="big")
                    psys.append(psy)
                    nc.tensor.matmul(
                        psy[:],
                        lhsT=og_tiles[(ib, 0)][:, 128 * ic:128 * ic + 128],
                        rhs=wo_sb[0][:], start=True, stop=False)
                for k, ic in enumerate((2 * icg, 2 * icg + 1)):
                    nc.tensor.matmul(
                        psys[k][:],
                        lhsT=og_tiles[(ib, 1)][:, 128 * ic:128 * ic + 128],
                        rhs=wo_sb[1][:], start=False, stop=True)
                for k, ic in enumerate((2 * icg, 2 * icg + 1)):
                    it = 4 * ib + ic
                    yt = small.tile([128, 256], FP16, tag="yt")
                    nc.scalar.copy(yt[:], psys[k][:])
                    nc.sync.dma_start(out=y_d[128 * it:128 * it + 128, :],
                                      in_=yt[:])
            for pr in range(2):
                del og_tiles[(ib, pr)]

    nsteps = len(steps)
    for s in range(nsteps + 1):
        cur = None
        if s < nsteps:
            ib, h = steps[s]
            pair, off = h // 2, 64 * (h % 2)
            if env.get("static_bt"):
                bt = env["static_bt"][s]
            else:
                bt = biasp.tile([128, NJP, 512], FP16, tag="bias")
                bsrc = bias_d[h, ib].rearrange("p (j n) -> p j n", j=NJP)
                nc.sync.dma_start(out=bt[:], in_=bsrc[:])
            poA_t = ps_o.tile([65, 512], F32, tag="po")
            poB_t = ps_o.tile([65, 512], F32, tag="po")
            cur = {"ib": ib, "h": h, "poA": poA_t, "poB": poB_t, "pts": []}
        for j in range(NJP):
            if cur is not None:
                jp = j // 2
                cls = jp_class(jp, h)
                if j % 2 == 0:
                    ps_t = ps_big.tile([128, 1024], F32, tag="big")
                    cur["ps"] = ps_t
                nc.tensor.matmul(
                    cur["ps"][:, 512 * (j % 2):512 * (j % 2) + 512],
                    lhsT=kT[pair][off:off + 64, 128 * j:128 * j + 128],
                    rhs=qT[pair][off:off + 64, 512 * ib:512 * ib + 512],
                    start=True, stop=(cls != "pe"))
                if j % 2 == 1:
                    if cls == "pe":
                        # add raw bias into S via identity matmuls; exp
                        # output is then the finished attention weight.
                        for jh in (j - 1, j):
                            nc.tensor.matmul(
                                cur["ps"][:, 512 * (jh % 2):512 * (jh % 2) + 512],
                                lhsT=ident[:],
                                rhs=bt[:, jh, :],
                                start=False, stop=True)
                    pe = pexp.tile([128, 2, 512], FP16, tag="pexp")
                    nc.scalar.activation(
                        pe[:].rearrange("p a n -> p (a n)"),
                        cur["ps"][:], AF.Exp)
                    if cls == "pe":
                        cur["pts"].append(pe)
                    else:
                        ptp = pmul.tile([128, 2, 512], FP16, tag="pmul")
                        cur["pts"].append(ptp)
                        eng = nc.vector if cls == "dve" else nc.gpsimd
                        eng.tensor_mul(ptp[:], pe[:], bt[:, j - 1:j + 1, :])
            for thunk in inter.get(s, {}).get(j, ()):
                thunk()
            if prev is not None:
                # 2-chain AV: slot j consumes jj=SLOT_JJ[j] so the Pool-mul
                # j-pair lands last; same-chain links stay 2 PE instructions
                # apart so the PSUM accumulate turnaround is hidden.
                jj = SLOT_JJ[j]
                ch = prev["poA"] if (j % 2) == 0 else prev["poB"]
                lk = j // 2
                nc.tensor.matmul(
                    ch[:],
                    lhsT=vaug[jj][:, prev["h"], :],
                    rhs=prev["pts"][jj // 2][:, jj % 2, :],
                    start=(lk == 0), stop=(lk == 3))
        if prev is not None:
            emit_tail(prev)
        prev = cur


_PROG = None


def _get_program():
    global _PROG
    if _PROG is None:
        _PROG = _build_program()
    return _PROG


def _prep_core_inputs(x, attn_bias, Wq, Wkv, Wo, Wg, bg, core):
    b, cp = core // 2, core % 2
    f16 = np.float16
    f32 = np.float32

    xt = np.ascontiguousarray(
        x[b].T.reshape(KK, 128, N).transpose(1, 0, 2)).astype(f16)

    hs = HPC * cp
    A = attn_bias[b, hs:hs + HPC]                      # [4, i, j]
    b5 = np.ascontiguousarray(
        A.reshape(HPC, NB, 512, NJP, 128).transpose(0, 1, 4, 3, 2)
    ).astype(f32, copy=False)                          # [h, ib, 128, NJP, 512]
    # j-tiles 0,1 (jp0) stay raw (added into S on PE); the rest ship exp'd
    # for the elementwise-multiply path.
    b5 = b5.copy()
    b5[:, :, :, 2:6, :] = np.exp(b5[:, :, :, 2:6, :])       # jp1, jp2
    b5[1::2, :, :, 6:8, :] = np.exp(b5[1::2, :, :, 6:8, :])  # jp3, odd heads
    bias_t = b5.reshape(HPC, NB, 128, NJP * 512).astype(f16)

    def wtile(w):   # [256, 256] -> [128, KK, 256] fp16
        return np.ascontiguousarray(
            w.reshape(KK, 128, 256).transpose(1, 0, 2)).astype(f16)

    wq_t = wtile(Wq[:, 256 * cp:256 * cp + 256] * SCALE)
    wk_t = wtile(Wkv[:, :INNER][:, 256 * cp:256 * cp + 256])
    wv_t = wtile(Wkv[:, INNER:][:, 256 * cp:256 * cp + 256])
    wg_t = wtile(Wg[:, 256 * cp:256 * cp + 256])

    g0 = 256 * cp
    bgh = np.zeros((64, HPC), f32)
    for h in range(HPC):
        bgh[:, h] = 0.5 * bg[g0 + 64 * h:g0 + 64 * h + 64]
    wo_t = np.ascontiguousarray(
        Wo[g0:g0 + 256, :].reshape(2, 128, 256)).astype(f16)

    return {
        "xt": xt, "bias_t": bias_t, "wq": wq_t, "wk": wk_t, "wv": wv_t,
        "wg": wg_t, "bgh": bgh, "wo": wo_t,
        "halves64": np.full((1, 64), 0.5, f16),
        "ident": np.eye(128, dtype=f16),
    }


_LAST_RESULTS = None


def kernel(x, attn_bias, Wq, Wkv, Wo, bo, Wg, bg, _trace=False, **_trace_kw):
    global _LAST_RESULTS
    x = np.asarray(x, np.float32)
    attn_bias = np.asarray(attn_bias, np.float32)
    Wq = np.asarray(Wq, np.float32)
    Wkv = np.asarray(Wkv, np.float32)
    Wo = np.asarray(Wo, np.float32)
    bo = np.asarray(bo, np.float32)
    Wg = np.asarray(Wg, np.float32)
    bg = np.asarray(bg, np.float32)

    nc = _get_program()
    in_maps = [_prep_core_inputs(x, attn_bias, Wq, Wkv, Wo, Wg, bg, c)
               for c in range(NCORES)]
    res = run_bass_kernel_spmd(nc, in_maps, list(range(NCORES)),
                               trace=_trace, **_trace_kw)
    _LAST_RESULTS = res

    y = np.empty((B, N, DIM), np.float32)
    for b in range(B):
        y[b] = (res.results[2 * b]["y"].astype(np.float32)
                + res.results[2 * b + 1]["y"].astype(np.float32) + bo)
    return y

